# revision 10
# baseline (speedup 1.0000x reference)
"""Trainium2 Bass kernel for a single attention head with query-axis softmax.

Reference semantics (per batch b):
    k = x @ Wk; q = x @ Wq; v = x @ Wv                 # [T, H]
    wei = (q @ k^T) * E**-0.5                          # [T(query), T(key)]
    wei = where(tril, wei, -inf)                       # causal: keep s <= t
    p = softmax(wei, axis=0 over query t)              # NOTE: query axis!
    out = p @ v                                        # [T, H]

Because the softmax normalizes over the query axis t (per key column s),
out[t,h] = sum_s E[t,s] * v[s,h] / d[s] with E[t,s] = exp(wei[t,s])
(zero for s > t) and d[s] = sum_t E[t,s].  The kernel computes E^T tiles
([s on partitions, t free]) so d is a free-axis row sum (fused into the
exp instruction via accum_out), scales v rows by 1/d, and accumulates
out^T on PE.  out^T is stored as-is; the host un-transposes during the
gather (free), so no on-device layout fixup is needed.

The causal triangle mask on the diagonal block is applied ON the PE:
a 128-row matmul against the identity writes the additive -1e30 triangle
into PSUM (start of the accumulation group), and the diagonal S matmul
accumulates on top of it - no vector/gpsimd op, no cross-engine hop.

Projections: k and q are packed into one 128-partition stationary
([kT; qT] stacked), v separate; one wide PSUM->SBUF cast per column
block moves all three.

Sharding: batch dim (8) across the 8 NeuronCores, weights replicated.
x is host-packed per column block ([NJ, 128, NE*CB] bf16) and the four
block DMAs are explicitly serialized so block 3 lands at full wire
bandwidth instead of sharing it 3 ways.
"""

import os

import numpy as np
import ml_dtypes

import concourse.bass as bass
import concourse.tile as tile
from concourse import bacc, mybir
from concourse import bass_utils
from concourse.tile import add_dep_helper

B, T, E, H = 8, 2048, 1024, 64
P = 128                       # partitions
CB = 512                      # column block (t) width
NE = E // P                   # 8 contraction chunks for projections
NJ = T // CB                  # 4 column blocks
SCALE = float(E) ** -0.5      # note: embed**-0.5, not head_size**-0.5
MASK_NEG = -1.0e30
F32 = mybir.dt.float32
BF16 = mybir.dt.bfloat16
X = mybir.AxisListType.X
EXP = mybir.ActivationFunctionType.Exp
COPY = mybir.ActivationFunctionType.Copy

# packed weights tensor column offsets (all bf16)
WKV0 = 0                      # [P, NE*P]   chunk e: [Wk_e | Wv_e]
WQ0 = WKV0 + NE * P           # [P, NE*H]
MASK0 = WQ0 + NE * H          # [P, 4*P]    additive -1e30 triangles
ID0 = MASK0 + 4 * P           # [P, P]      identity
WALLW = ID0 + P


def _emit(tc, xb_d, wall_d, out_d):
    nc = tc.nc
    from contextlib import ExitStack

    with ExitStack() as ctx:
        singles = ctx.enter_context(tc.tile_pool(name="singles", bufs=1))
        epool = ctx.enter_context(tc.tile_pool(name="erow", bufs=9))
        dpool = ctx.enter_context(tc.tile_pool(name="dsmall", bufs=12))
        vpool = ctx.enter_context(tc.tile_pool(name="vrow", bufs=9))
        ps = ctx.enter_context(tc.tile_pool(name="ps", bufs=2, space="PSUM"))
        pproj_pool = ctx.enter_context(
            tc.tile_pool(name="pproj", bufs=1, space="PSUM")
        )
        pout = ctx.enter_context(tc.tile_pool(name="pout", bufs=1, space="PSUM"))

        # all weights/masks/identity in ONE DMA on the scalar queue
        wall = singles.tile([P, WALLW], BF16, name="wall")
        nc.scalar.dma_start(out=wall[:], in_=wall_d[:])
        wkv = wall[:, WKV0 : WKV0 + NE * P]
        wq = wall[:, WQ0 : WQ0 + NE * H]
        masks = wall[:, MASK0 : MASK0 + 4 * P]
        identb = wall[:, ID0 : ID0 + P]

        # x block DMAs on the sync queue, serialized j=3 -> 0 so each gets
        # the full wire instead of sharing it 4 ways
        xts = {}
        prev_dma = None
        for j in reversed(range(NJ)):
            xt = singles.tile([P, NE * CB], BF16, name=f"x{j}")
            d = nc.sync.dma_start(out=xt[:], in_=xb_d[j])
            if prev_dma is not None:
                add_dep_helper(
                    prev_dma.ins, d.ins, sync=True, reason="serialize x dma"
                )
            prev_dma = d
            xts[j] = xt

        # ~4us of dummy matmuls while the first DMAs land: primes the PE
        # activity monitor so the real chains start at 2.4 GHz, not 1.2
        junk = singles.tile([P, CB], BF16)
        nc.gpsimd.memset(junk[:], 1.0)
        pwarm = ps.tile([P, 2 * CB], F32, tag="ps", name="pwarm")
        for w in range(10):
            nc.tensor.matmul(
                pwarm[:, 0:CB],
                lhsT=junk[:, 0:P],
                rhs=junk[:],
                start=(w == 0),
                stop=(w == 9),
            )

        # per-block projected activations: [kT(0:64); vT(64:128)] cols 0:CB,
        # [qT(0:64); junk] cols CB:2CB (k and q share partition base 0 so
        # the S matmul's lhsT/rhs bases match)
        kqv = {
            j: singles.tile([P, 2 * CB], BF16, name=f"kqv{j}")
            for j in range(NJ)
        }

        # out^T accumulators packed 2 per bank: jj even rows 0:64, odd 64:128.
        # Accumulation groups on disjoint partition ranges of one bank are
        # fine on HW (per-element has_written); skip the sim's coarse check.
        pout_tiles = [
            pout.tile([P, CB], F32, tag=f"pt{a}", name=f"pt{a}") for a in range(2)
        ]
        outst = singles.tile([P, 2 * CB], F32, name="outst")

        def pout_slice(jj, c0, c1):
            rb = H * (jj % 2)
            return pout_tiles[jj // 2][rb : rb + H, c0:c1]

        # deferred AV emission (lag behind S so PE never waits on the
        # d / v' chain): each entry = (r, j_of_row), d0, erow, vi
        pending_av = []

        def _av_one(rj, d0, erow, vi, jj):
            c = (jj - rj[1]) * CB
            lo = d0 if jj == rj[1] else 0
            nc.tensor.matmul(
                pout_slice(jj, lo, CB),
                lhsT=vi[:],
                rhs=erow[:, c + lo : c + CB],
                start=(jj == rj[1] and rj[0] == 0),
                stop=(rj[1] == 0 and rj[0] == 3),
                skip_group_check=True,
            )

        def close_bank(a):
            # stage out^T bank a PSUM->SBUF (vector for bank 0, scalar for
            # bank 1 so the two copies overlap), then store it
            half = outst[:, a * CB : (a + 1) * CB]
            if a == 0:
                nc.vector.tensor_copy(half, pout_tiles[0][:])
            else:
                nc.scalar.activation(out=half, in_=pout_tiles[1][:], func=COPY)
            nc.sync.dma_start(
                out=out_d[:, a * CB : (a + 1) * CB], in_=half
            )

        def flush_av(final):
            if final:
                # group by output bank so bank A closes early and its copy
                # and store overlap bank B's last matmuls
                rows = list(pending_av)
                pending_av.clear()
                for jj in range(NJ):
                    for rj, d0, erow, vi in rows:
                        if jj >= rj[1]:
                            _av_one(rj, d0, erow, vi, jj)
                    if jj % 2 == 1:
                        close_bank(jj // 2)
                return
            rj, d0, erow, vi = pending_av.pop(0)
            for jj in range(rj[1], NJ):
                _av_one(rj, d0, erow, vi, jj)

        # projection matmul emission is spread through the PREVIOUS step's
        # rows so the PE instruction stream stays dense (HAM stays warm)
        def proj_thunks(j):
            pproj = pproj_pool.tile([P, 2 * CB], F32, tag="pp", name="pproj")
            thunks = []
            for e in range(NE):
                thunks.append(
                    lambda e=e: nc.tensor.matmul(
                        pproj[:, 0:CB],
                        lhsT=wkv[:, e * P : (e + 1) * P],
                        rhs=xts[j][:, e * CB : (e + 1) * CB],
                        start=(e == 0),
                        stop=(e == NE - 1),
                    )
                )
            for e in range(NE):
                thunks.append(
                    lambda e=e: nc.tensor.matmul(
                        pproj[0:H, CB : 2 * CB],
                        lhsT=wq[:, e * H : (e + 1) * H],
                        rhs=xts[j][:, e * CB : (e + 1) * CB],
                        start=(e == 0),
                        stop=(e == NE - 1),
                    )
                )
            return pproj, thunks

        def proj_cast(j, pproj, engine):
            # one wide PSUM->SBUF bf16 cast for kT, qT and vT together
            if engine == "scalar":
                nc.scalar.activation(out=kqv[j][:], in_=pproj[:], func=COPY)
            else:
                nc.vector.tensor_copy(kqv[j][:], pproj[:])

        # --- main pipeline: column blocks in descending order --------------
        next_proj = []  # pending matmul thunks for step j-1's projections

        def drip_proj(k):
            for _ in range(min(k, len(next_proj))):
                next_proj.pop(0)()

        pproj, thunks = proj_thunks(3)
        for t in thunks:
            t()
        proj_cast(3, pproj, "scalar")

        for j in reversed(range(NJ)):
            if j > 0:
                pproj_next, next_proj = proj_thunks(j - 1)

            # rows i = 4j .. 4j+3 of E^T are now computable in full
            for r in range(4):
                i = 4 * j + r
                d0 = r * P  # local offset of this s-chunk within block j
                nblk = NJ - j
                npair = (nblk + 1) // 2
                erow = epool.tile([P, T], BF16)
                dparts = dpool.tile([P, 2], F32, tag="dparts")
                kT_sl = kqv[j][0:H, d0 : d0 + P]

                psts = []
                for pair in range(npair):
                    jj0 = j + 2 * pair
                    w = CB * min(2, NJ - jj0)  # 512 or 1024
                    if j == 0 and pair == 1:
                        # projection pool is idle during the last step
                        pst = pproj_pool.tile(
                            [P, 2 * CB], F32, tag="pp", name="pst0"
                        )
                    else:
                        pst = ps.tile([P, 2 * CB], F32, tag="ps")
                    psts.append((pst, jj0, w))

                # additive -1e30 triangle into PSUM via the PE (identity
                # stationary), then all S matmuls back-to-back with the
                # same kT stationary (single weight load with ldw-opt)
                pst0 = psts[0][0]
                nc.tensor.matmul(
                    pst0[:, d0 : d0 + P],
                    lhsT=identb,
                    rhs=masks[:, r * P : (r + 1) * P],
                    start=True,
                    stop=False,
                )
                for pst, jj0, w in psts:
                    for u in range(w // CB):
                        jj = jj0 + u
                        c0 = u * CB
                        if jj == j:
                            # diagonal block: accumulate onto the mask,
                            # then the clean remainder of the block
                            nc.tensor.matmul(
                                pst[:, d0 : d0 + P],
                                lhsT=kT_sl,
                                rhs=kqv[jj][0:H, CB + d0 : CB + d0 + P],
                                start=False,
                                stop=True,
                            )
                            if d0 + P < CB:
                                nc.tensor.matmul(
                                    pst[:, d0 + P : CB],
                                    lhsT=kT_sl,
                                    rhs=kqv[jj][0:H, CB + d0 + P : 2 * CB],
                                    start=True,
                                    stop=True,
                                )
                        else:
                            nc.tensor.matmul(
                                pst[:, c0 : c0 + CB],
                                lhsT=kT_sl,
                                rhs=kqv[jj][0:H, CB : 2 * CB],
                                start=True,
                                stop=True,
                            )
                drip_proj(3)

                # exp (+ d partial sums fused via accum_out), per pair
                for pair, (pst, jj0, w) in enumerate(psts):
                    lo = d0 if pair == 0 else 0
                    c = 2 * CB * pair
                    nc.scalar.activation(
                        out=erow[:, c + lo : c + w],
                        in_=pst[:, lo:w],
                        func=EXP,
                        scale=SCALE,
                        accum_out=dparts[:, pair : pair + 1],
                    )

                # d = sum over the row; 1/d feeds the v' scale
                dinv = dpool.tile([P, 1], F32, tag="dinv")
                if npair > 1:
                    dsum = dpool.tile([P, 1], F32, tag="dsum")
                    nc.vector.reduce_sum(dsum[:], dparts[:, 0:npair], axis=X)
                    nc.vector.reciprocal(dinv[:], dsum[:])
                else:
                    nc.vector.reciprocal(dinv[:], dparts[:, 0:1])

                pvt = ps.tile([P, 2 * CB], BF16, tag="ps", name="pvt")
                nc.tensor.transpose(
                    pvt[:, 0:H],
                    kqv[j][H:P, d0 : d0 + P],
                    identb[H:P, H:P],
                )
                vi = vpool.tile([P, H], BF16, tag="vi", name="vi")
                nc.vector.tensor_scalar_mul(vi[:], pvt[:, 0:H], dinv[:])

                lag = 4 if j == 1 else (1 if j == 0 else 2)
                while len(pending_av) >= lag:
                    flush_av(False)  # AV matmuls lag behind S for overlap
                drip_proj(2)
                pending_av.append(((r, j), d0, erow, vi))

            # drain remaining next-step projection matmuls, then its cast
            drip_proj(len(next_proj))
            if j > 0:
                proj_cast(j - 1, pproj_next, "vector" if j == 1 else "scalar")

        flush_av(True)


def _enable_ldw_opt():
    """Flip walrus's --enable-ldw-opt to true for our compile: consecutive
    matmuls reusing the same stationary operand then skip the reload."""
    import concourse.bass_utils as bu

    if getattr(bu, "_ldw_opt_patched", False):
        return
    orig = bu.run_command

    def run_command_ldw(cmd, *a, **kw):
        if isinstance(cmd, list):
            cmd = [
                "--enable-ldw-opt=true" if c == "--enable-ldw-opt=false" else c
                for c in cmd
            ]
        return orig(cmd, *a, **kw)

    bu.run_command = run_command_ldw
    bu._ldw_opt_patched = True


def _build_program():
    # NOTE: walrus rejects --enable-ldw-opt=true when the program contains
    # transpose-mode LDWEIGHTS (our pvt transposes), so this stays opt-in.
    if os.environ.get("BASS_LDW_OPT", "0") == "1":
        _enable_ldw_opt()
    nc = bacc.Bacc("TRN2", target_bir_lowering=False, debug=False, num_devices=B)
    xb_d = nc.dram_tensor("xb", [NJ, P, NE * CB], BF16, kind="ExternalInput").ap()
    wall_d = nc.dram_tensor("wall", [P, WALLW], BF16, kind="ExternalInput").ap()
    out_d = nc.dram_tensor("out", [P, 2 * CB], F32, kind="ExternalOutput").ap()
    with tile.TileContext(nc) as tc:
        _emit(tc, xb_d, wall_d, out_d)
    nc.compile()
    return nc


def _host_masks():
    """[128, 4*128] additive triangles: row r masks t < s within the
    diagonal 128-block (t-local f, partition p: keep f >= p)."""
    m = np.full((P, 4 * P), MASK_NEG, dtype=np.float32)
    p = np.arange(P)[:, None]
    f = np.arange(P)[None, :]
    for r in range(4):
        m[:, r * P : (r + 1) * P][f >= p] = 0.0
    return m


def _host_inputs(x, Wk, Wq, Wv):
    bf = ml_dtypes.bfloat16
    x = np.asarray(x, dtype=np.float32)
    # [B, E, T] -> block-major [B, NJ, P, NE*CB] so each block is one
    # contiguous DMA with 4KB/partition runs
    xT = np.transpose(x, (0, 2, 1)).reshape(B, NE, P, NJ, CB)
    xb = np.ascontiguousarray(xT.transpose(0, 3, 2, 1, 4)).reshape(
        B, NJ, P, NE * CB
    ).astype(bf)

    def chunks(w):  # [E, h] -> [NE, P, h]
        return np.asarray(w, np.float32).reshape(NE, P, -1)

    # wkv chunk e = [Wk_e | Wv_e] -> [P, NE*128]
    kv = np.concatenate([chunks(Wk), chunks(Wv)], axis=2)
    wkv = kv.transpose(1, 0, 2).reshape(P, NE * P)
    wqp = chunks(Wq).transpose(1, 0, 2).reshape(P, NE * H)
    wall = np.concatenate(
        [wkv, wqp, _host_masks(), np.eye(P, dtype=np.float32)], axis=1
    ).astype(bf)
    assert wall.shape == (P, WALLW)
    return [{"xb": xb[b], "wall": wall} for b in range(B)]


def _unpack_out(outT):
    """[128, 1024] out^T banks -> [T, H] natural layout."""
    o = np.empty((T, H), dtype=np.float32)
    for a in range(2):
        for h2 in range(2):
            jj = 2 * a + h2
            o[jj * CB : (jj + 1) * CB, :] = outT[
                H * h2 : H * (h2 + 1), a * CB : (a + 1) * CB
            ].T
    return o


def _ensure_axon_ntff_hook():
    """The agent image's antenv lacks axon_hooks; synthesize it so
    run_bass_kernel_spmd's trace path can find the NTFF profile hook."""
    import sys
    import types

    if "antenv.axon_hooks" in sys.modules:
        return
    try:
        import antenv

        mod = types.ModuleType("antenv.axon_hooks")
        mod._hook = None

        def set_axon_ntff_profile_hook(h):
            mod._hook = h

        def get_axon_ntff_profile_hook():
            return mod._hook

        mod.set_axon_ntff_profile_hook = set_axon_ntff_profile_hook
        mod.get_axon_ntff_profile_hook = get_axon_ntff_profile_hook
        sys.modules["antenv.axon_hooks"] = mod
        antenv.axon_hooks = mod

        from trn_agent_boot.trn_boot import _ntff_profile_via_ctypes

        hook = _ntff_profile_via_ctypes("/opt/axon/libaxon_pjrt.so")
        if hook is not None:
            mod._hook = hook
    except Exception as e:  # degrade to untraced run
        print(f"NTFF hook setup failed ({e}); tracing will be skipped")


def kernel(x, Wk, Wq, Wv, _trace=False, _trace_kwargs=None):
    if _trace:
        _ensure_axon_ntff_hook()
    in_maps = _host_inputs(x, Wk, Wq, Wv)
    nc = _build_program()
    res = bass_utils.run_bass_kernel_spmd(
        nc, in_maps, list(range(B)), trace=_trace, **(_trace_kwargs or {})
    )
    out = np.stack(
        [_unpack_out(res.results[b]["out"]) for b in range(B)], axis=0
    )
    if _trace:
        kernel.last_results = res
    return out.astype(np.float32)


# revision 13
# speedup vs baseline: 1.0922x; 1.0922x over previous
"""Trainium2 Bass kernel for a single attention head with query-axis softmax.

Reference semantics (per batch b):
    k = x @ Wk; q = x @ Wq; v = x @ Wv                 # [T, H]
    wei = (q @ k^T) * E**-0.5                          # [T(query), T(key)]
    wei = where(tril, wei, -inf)                       # causal: keep s <= t
    p = softmax(wei, axis=0 over query t)              # NOTE: query axis!
    out = p @ v                                        # [T, H]

Because the softmax normalizes over the query axis t (per key column s),
out[t,h] = sum_s E[t,s] * v[s,h] / d[s] with E[t,s] = exp(wei[t,s])
(zero for s > t) and d[s] = sum_t E[t,s].  The kernel computes E^T tiles
([s on partitions, t free]) so d is a free-axis row sum (fused into the
exp instruction via accum_out), scales v rows by 1/d, and accumulates
out^T on PE.  out^T is stored as-is; the host un-transposes during the
gather (free), so no on-device layout fixup is needed.

The causal triangle mask on the diagonal block is applied ON the PE:
a 128-row matmul against the identity writes the additive -1e30 triangle
into PSUM (start of the accumulation group), and the diagonal S matmul
accumulates on top of it - no vector/gpsimd op, no cross-engine hop.

Projections: k and q are packed into one 128-partition stationary
([kT; qT] stacked), v separate; one wide PSUM->SBUF cast per column
block moves all three.

Sharding: batch dim (8) across the 8 NeuronCores, weights replicated.
x is host-packed per column block ([NJ, 128, NE*CB] bf16) and the four
block DMAs are explicitly serialized so block 3 lands at full wire
bandwidth instead of sharing it 3 ways.
"""

import os

import numpy as np
import ml_dtypes

import concourse.bass as bass
import concourse.tile as tile
from concourse import bacc, mybir
from concourse import bass_utils
B, T, E, H = 8, 2048, 1024, 64
P = 128                       # partitions
CB = 512                      # column block (t) width
NE = E // P                   # 8 contraction chunks for projections
NJ = T // CB                  # 4 column blocks
SCALE = float(E) ** -0.5      # note: embed**-0.5, not head_size**-0.5
MASK_NEG = -1.0e30
F32 = mybir.dt.float32
BF16 = mybir.dt.bfloat16
X = mybir.AxisListType.X
EXP = mybir.ActivationFunctionType.Exp
COPY = mybir.ActivationFunctionType.Copy

# packed weights tensor column offsets (all bf16)
WKV0 = 0                      # [P, NE*P]   chunk e: [Wk_e | Wv_e]
WQ0 = WKV0 + NE * P           # [P, NE*H]
MASK0 = WQ0 + NE * H          # [P, 4*P]    additive -1e30 triangles
ID0 = MASK0 + 4 * P           # [P, P]      identity
WALLW = ID0 + P


def _emit(tc, xb_d, wall_d, out_d):
    nc = tc.nc
    from contextlib import ExitStack

    with ExitStack() as ctx:
        singles = ctx.enter_context(tc.tile_pool(name="singles", bufs=1))
        epool = ctx.enter_context(tc.tile_pool(name="erow", bufs=9))
        dpool = ctx.enter_context(tc.tile_pool(name="dsmall", bufs=12))
        vpool = ctx.enter_context(tc.tile_pool(name="vrow", bufs=9))
        ps = ctx.enter_context(tc.tile_pool(name="ps", bufs=2, space="PSUM"))
        pproj_pool = ctx.enter_context(
            tc.tile_pool(name="pproj", bufs=1, space="PSUM")
        )
        pout = ctx.enter_context(tc.tile_pool(name="pout", bufs=1, space="PSUM"))

        # all weights/masks/identity in ONE DMA on the scalar queue
        wall = singles.tile([P, WALLW], BF16, name="wall")
        nc.scalar.dma_start(out=wall[:], in_=wall_d[:])
        wkv = wall[:, WKV0 : WKV0 + NE * P]
        wq = wall[:, WQ0 : WQ0 + NE * H]
        masks = wall[:, MASK0 : MASK0 + 4 * P]
        identb = wall[:, ID0 : ID0 + P]

        # x blocks: one dma_start tops out at ~165 GB/s, so each block is
        # split in half across two queues (sync + gpsimd) for two
        # concurrent streams; later blocks are paced behind earlier ones
        # by 1-element WAW fences on the gpsimd queue (the fence reads
        # block j+1, so it waits for its transfer, and writes into block
        # j's tile, so both of block j's half-DMAs wait on the fence)
        xts = {
            j: singles.tile([P, NE * CB], BF16, name=f"x{j}")
            for j in range(NJ)
        }
        junk = singles.tile([P, CB], BF16)
        nc.gpsimd.memset(junk[:], 1.0)
        HALF = NE * CB // 2
        nc.sync.dma_start(out=xts[3][:, 0:HALF], in_=xb_d[3][:, 0:HALF])
        nc.gpsimd.dma_start(out=xts[3][:, HALF:], in_=xb_d[3][:, HALF:])
        for j in (2, 1, 0):
            nc.gpsimd.tensor_copy(xts[j][0:1, 0:1], xts[j + 1][0:1, 0:1])
            nc.sync.dma_start(out=xts[j][:, 0:HALF], in_=xb_d[j][:, 0:HALF])
            nc.gpsimd.dma_start(out=xts[j][:, HALF:], in_=xb_d[j][:, HALF:])

        # ~4us of dummy matmuls while the first DMAs land: primes the PE
        # activity monitor so the real chains start at 2.4 GHz, not 1.2
        pwarm = ps.tile([P, 2 * CB], F32, tag="ps", name="pwarm")
        for w in range(10):
            nc.tensor.matmul(
                pwarm[:, 0:CB],
                lhsT=junk[:, 0:P],
                rhs=junk[:],
                start=(w == 0),
                stop=(w == 9),
            )

        # per-block projected activations: [kT(0:64); vT(64:128)] cols 0:CB,
        # [qT(0:64); junk] cols CB:2CB (k and q share partition base 0 so
        # the S matmul's lhsT/rhs bases match)
        kqv = {
            j: singles.tile([P, 2 * CB], BF16, name=f"kqv{j}")
            for j in range(NJ)
        }

        # out^T accumulators packed 2 per bank: jj even rows 0:64, odd 64:128.
        # Accumulation groups on disjoint partition ranges of one bank are
        # fine on HW (per-element has_written); skip the sim's coarse check.
        pout_tiles = [
            pout.tile([P, CB], F32, tag=f"pt{a}", name=f"pt{a}") for a in range(2)
        ]
        outst = singles.tile([P, 2 * CB], F32, name="outst")

        def pout_slice(jj, c0, c1):
            rb = H * (jj % 2)
            return pout_tiles[jj // 2][rb : rb + H, c0:c1]

        # deferred AV emission (lag behind S so PE never waits on the
        # d / v' chain): each entry = (r, j_of_row), d0, erow, vi
        pending_av = []

        def _av_one(rj, d0, erow, vi, jj):
            c = (jj - rj[1]) * CB
            lo = d0 if jj == rj[1] else 0
            nc.tensor.matmul(
                pout_slice(jj, lo, CB),
                lhsT=vi[:],
                rhs=erow[:, c + lo : c + CB],
                start=(jj == rj[1] and rj[0] == 0),
                stop=(rj[1] == 0 and rj[0] == 3),
                skip_group_check=True,
            )

        def close_bank(a):
            # stage out^T bank a PSUM->SBUF (vector for bank 0, scalar for
            # bank 1 so the two copies overlap), then store it
            half = outst[:, a * CB : (a + 1) * CB]
            if a == 0:
                nc.vector.tensor_copy(half, pout_tiles[0][:])
            else:
                nc.scalar.activation(out=half, in_=pout_tiles[1][:], func=COPY)
            nc.sync.dma_start(
                out=out_d[:, a * CB : (a + 1) * CB], in_=half
            )

        def flush_av(final):
            if final:
                # group by output bank so bank A closes early and its copy
                # and store overlap bank B's last matmuls
                rows = list(pending_av)
                pending_av.clear()
                for jj in range(NJ):
                    for rj, d0, erow, vi in rows:
                        if jj >= rj[1]:
                            _av_one(rj, d0, erow, vi, jj)
                    if jj % 2 == 1:
                        close_bank(jj // 2)
                return
            rj, d0, erow, vi = pending_av.pop(0)
            for jj in range(rj[1], NJ):
                _av_one(rj, d0, erow, vi, jj)

        # projection matmul emission is spread through the PREVIOUS step's
        # rows so the PE instruction stream stays dense (HAM stays warm)
        def proj_thunks(j):
            pproj = pproj_pool.tile([P, 2 * CB], F32, tag="pp", name="pproj")
            thunks = []
            for e in range(NE):
                thunks.append(
                    lambda e=e: nc.tensor.matmul(
                        pproj[:, 0:CB],
                        lhsT=wkv[:, e * P : (e + 1) * P],
                        rhs=xts[j][:, e * CB : (e + 1) * CB],
                        start=(e == 0),
                        stop=(e == NE - 1),
                    )
                )
            for e in range(NE):
                thunks.append(
                    lambda e=e: nc.tensor.matmul(
                        pproj[0:H, CB : 2 * CB],
                        lhsT=wq[:, e * H : (e + 1) * H],
                        rhs=xts[j][:, e * CB : (e + 1) * CB],
                        start=(e == 0),
                        stop=(e == NE - 1),
                    )
                )
            return pproj, thunks

        def proj_cast(j, pproj):
            # one wide PSUM->SBUF bf16 cast for kT, qT and vT together
            # (vector, so the scalar engine stays dedicated to the exps)
            nc.vector.tensor_copy(kqv[j][:], pproj[:])

        # --- main pipeline: column blocks in descending order --------------
        next_proj = []  # pending matmul thunks for step j-1's projections

        def drip_proj(k):
            for _ in range(min(k, len(next_proj))):
                next_proj.pop(0)()

        pproj, thunks = proj_thunks(3)
        for t in thunks:
            t()
        proj_cast(3, pproj)

        for j in reversed(range(NJ)):
            if j > 0:
                pproj_next, next_proj = proj_thunks(j - 1)

            # rows i = 4j .. 4j+3 of E^T are now computable in full
            for r in range(4):
                i = 4 * j + r
                d0 = r * P  # local offset of this s-chunk within block j
                nblk = NJ - j
                npair = (nblk + 1) // 2
                erow = epool.tile([P, T], BF16)
                dparts = dpool.tile([P, 2], F32, tag="dparts")
                kT_sl = kqv[j][0:H, d0 : d0 + P]

                psts = []
                for pair in range(npair):
                    jj0 = j + 2 * pair
                    w = CB * min(2, NJ - jj0)  # 512 or 1024
                    if j == 0 and pair == 1:
                        # projection pool is idle during the last step
                        pst = pproj_pool.tile(
                            [P, 2 * CB], F32, tag="pp", name="pst0"
                        )
                    else:
                        pst = ps.tile([P, 2 * CB], F32, tag="ps")
                    psts.append((pst, jj0, w))

                # additive -1e30 triangle into PSUM via the PE (identity
                # stationary), then all S matmuls back-to-back with the
                # same kT stationary (single weight load with ldw-opt)
                pst0 = psts[0][0]
                nc.tensor.matmul(
                    pst0[:, d0 : d0 + P],
                    lhsT=identb,
                    rhs=masks[:, r * P : (r + 1) * P],
                    start=True,
                    stop=False,
                )
                for pst, jj0, w in psts:
                    for u in range(w // CB):
                        jj = jj0 + u
                        c0 = u * CB
                        if jj == j:
                            # diagonal block: accumulate onto the mask,
                            # then the clean remainder of the block
                            nc.tensor.matmul(
                                pst[:, d0 : d0 + P],
                                lhsT=kT_sl,
                                rhs=kqv[jj][0:H, CB + d0 : CB + d0 + P],
                                start=False,
                                stop=True,
                            )
                            if d0 + P < CB:
                                nc.tensor.matmul(
                                    pst[:, d0 + P : CB],
                                    lhsT=kT_sl,
                                    rhs=kqv[jj][0:H, CB + d0 + P : 2 * CB],
                                    start=True,
                                    stop=True,
                                )
                        else:
                            nc.tensor.matmul(
                                pst[:, c0 : c0 + CB],
                                lhsT=kT_sl,
                                rhs=kqv[jj][0:H, CB : 2 * CB],
                                start=True,
                                stop=True,
                            )
                drip_proj(3)

                # exp (+ d partial sums fused via accum_out), per pair
                for pair, (pst, jj0, w) in enumerate(psts):
                    lo = d0 if pair == 0 else 0
                    c = 2 * CB * pair
                    nc.scalar.activation(
                        out=erow[:, c + lo : c + w],
                        in_=pst[:, lo:w],
                        func=EXP,
                        scale=SCALE,
                        accum_out=dparts[:, pair : pair + 1],
                    )

                # d = sum over the row; 1/d feeds the v' scale
                dinv = dpool.tile([P, 1], F32, tag="dinv")
                if npair > 1:
                    dsum = dpool.tile([P, 1], F32, tag="dsum")
                    nc.vector.reduce_sum(dsum[:], dparts[:, 0:npair], axis=X)
                    nc.vector.reciprocal(dinv[:], dsum[:])
                else:
                    nc.vector.reciprocal(dinv[:], dparts[:, 0:1])

                pvt = ps.tile([P, 2 * CB], BF16, tag="ps", name="pvt")
                nc.tensor.transpose(
                    pvt[:, 0:H],
                    kqv[j][H:P, d0 : d0 + P],
                    identb[H:P, H:P],
                )
                vi = vpool.tile([P, H], BF16, tag="vi", name="vi")
                nc.vector.tensor_scalar_mul(vi[:], pvt[:, 0:H], dinv[:])

                lag = 4 if j == 1 else (1 if j == 0 else 2)
                while len(pending_av) >= lag:
                    flush_av(False)  # AV matmuls lag behind S for overlap
                drip_proj(2)
                pending_av.append(((r, j), d0, erow, vi))

            # drain remaining next-step projection matmuls, then its cast
            drip_proj(len(next_proj))
            if j > 0:
                proj_cast(j - 1, pproj_next)

        flush_av(True)


def _enable_ldw_opt():
    """Flip walrus's --enable-ldw-opt to true for our compile: consecutive
    matmuls reusing the same stationary operand then skip the reload."""
    import concourse.bass_utils as bu

    if getattr(bu, "_ldw_opt_patched", False):
        return
    orig = bu.run_command

    def run_command_ldw(cmd, *a, **kw):
        if isinstance(cmd, list):
            cmd = [
                "--enable-ldw-opt=true" if c == "--enable-ldw-opt=false" else c
                for c in cmd
            ]
        return orig(cmd, *a, **kw)

    bu.run_command = run_command_ldw
    bu._ldw_opt_patched = True


def _build_program():
    # NOTE: walrus rejects --enable-ldw-opt=true when the program contains
    # transpose-mode LDWEIGHTS (our pvt transposes), so this stays opt-in.
    if os.environ.get("BASS_LDW_OPT", "0") == "1":
        _enable_ldw_opt()
    nc = bacc.Bacc("TRN2", target_bir_lowering=False, debug=False, num_devices=B)
    xb_d = nc.dram_tensor("xb", [NJ, P, NE * CB], BF16, kind="ExternalInput").ap()
    wall_d = nc.dram_tensor("wall", [P, WALLW], BF16, kind="ExternalInput").ap()
    out_d = nc.dram_tensor("out", [P, 2 * CB], F32, kind="ExternalOutput").ap()
    with tile.TileContext(nc) as tc:
        _emit(tc, xb_d, wall_d, out_d)
    nc.compile()
    return nc


def _host_masks():
    """[128, 4*128] additive triangles: row r masks t < s within the
    diagonal 128-block (t-local f, partition p: keep f >= p)."""
    m = np.full((P, 4 * P), MASK_NEG, dtype=np.float32)
    p = np.arange(P)[:, None]
    f = np.arange(P)[None, :]
    for r in range(4):
        m[:, r * P : (r + 1) * P][f >= p] = 0.0
    return m


def _host_inputs(x, Wk, Wq, Wv):
    bf = ml_dtypes.bfloat16
    x = np.asarray(x, dtype=np.float32)
    # [B, E, T] -> block-major [B, NJ, P, NE*CB] so each block is one
    # contiguous DMA with 4KB/partition runs
    xT = np.transpose(x, (0, 2, 1)).reshape(B, NE, P, NJ, CB)
    xb = np.ascontiguousarray(xT.transpose(0, 3, 2, 1, 4)).reshape(
        B, NJ, P, NE * CB
    ).astype(bf)

    def chunks(w):  # [E, h] -> [NE, P, h]
        return np.asarray(w, np.float32).reshape(NE, P, -1)

    # wkv chunk e = [Wk_e | Wv_e] -> [P, NE*128]
    kv = np.concatenate([chunks(Wk), chunks(Wv)], axis=2)
    wkv = kv.transpose(1, 0, 2).reshape(P, NE * P)
    wqp = chunks(Wq).transpose(1, 0, 2).reshape(P, NE * H)
    wall = np.concatenate(
        [wkv, wqp, _host_masks(), np.eye(P, dtype=np.float32)], axis=1
    ).astype(bf)
    assert wall.shape == (P, WALLW)
    return [{"xb": xb[b], "wall": wall} for b in range(B)]


def _unpack_out(outT):
    """[128, 1024] out^T banks -> [T, H] natural layout."""
    o = np.empty((T, H), dtype=np.float32)
    for a in range(2):
        for h2 in range(2):
            jj = 2 * a + h2
            o[jj * CB : (jj + 1) * CB, :] = outT[
                H * h2 : H * (h2 + 1), a * CB : (a + 1) * CB
            ].T
    return o


def _ensure_axon_ntff_hook():
    """The agent image's antenv lacks axon_hooks; synthesize it so
    run_bass_kernel_spmd's trace path can find the NTFF profile hook."""
    import sys
    import types

    if "antenv.axon_hooks" in sys.modules:
        return
    try:
        import antenv

        mod = types.ModuleType("antenv.axon_hooks")
        mod._hook = None

        def set_axon_ntff_profile_hook(h):
            mod._hook = h

        def get_axon_ntff_profile_hook():
            return mod._hook

        mod.set_axon_ntff_profile_hook = set_axon_ntff_profile_hook
        mod.get_axon_ntff_profile_hook = get_axon_ntff_profile_hook
        sys.modules["antenv.axon_hooks"] = mod
        antenv.axon_hooks = mod

        from trn_agent_boot.trn_boot import _ntff_profile_via_ctypes

        hook = _ntff_profile_via_ctypes("/opt/axon/libaxon_pjrt.so")
        if hook is not None:
            mod._hook = hook
    except Exception as e:  # degrade to untraced run
        print(f"NTFF hook setup failed ({e}); tracing will be skipped")


def kernel(x, Wk, Wq, Wv, _trace=False, _trace_kwargs=None):
    if _trace:
        _ensure_axon_ntff_hook()
    in_maps = _host_inputs(x, Wk, Wq, Wv)
    nc = _build_program()
    res = bass_utils.run_bass_kernel_spmd(
        nc, in_maps, list(range(B)), trace=_trace, **(_trace_kwargs or {})
    )
    out = np.stack(
        [_unpack_out(res.results[b]["out"]) for b in range(B)], axis=0
    )
    if _trace:
        kernel.last_results = res
    return out.astype(np.float32)


# revision 15
# speedup vs baseline: 1.2941x; 1.1848x over previous
"""Trainium2 Bass kernel for a single attention head with query-axis softmax.

Reference semantics (per batch b):
    k = x @ Wk; q = x @ Wq; v = x @ Wv                 # [T, H]
    wei = (q @ k^T) * E**-0.5                          # [T(query), T(key)]
    wei = where(tril, wei, -inf)                       # causal: keep s <= t
    p = softmax(wei, axis=0 over query t)              # NOTE: query axis!
    out = p @ v                                        # [T, H]

Because the softmax normalizes over the query axis t (per key column s),
out[t,h] = sum_s E[t,s] * v[s,h] / d[s] with E[t,s] = exp(wei[t,s])
(zero for s > t) and d[s] = sum_t E[t,s].  The kernel computes E^T tiles
([s on partitions, t free]) so d is a free-axis row sum (fused into the
exp instruction via accum_out), scales v rows by 1/d, and accumulates
out^T on PE.  out^T is stored as-is; the host un-transposes during the
gather (free), so no on-device layout fixup is needed.

The causal triangle mask on the diagonal block is applied ON the PE:
a 128-row matmul against the identity writes the additive -1e30 triangle
into PSUM (start of the accumulation group), and the diagonal S matmul
accumulates on top of it - no vector/gpsimd op, no cross-engine hop.

Projections: k and q are packed into one 128-partition stationary
([kT; qT] stacked), v separate; one wide PSUM->SBUF cast per column
block moves all three.

Sharding: batch dim (8) across the 8 NeuronCores, weights replicated.
x is host-packed per column block ([NJ, 128, NE*CB] bf16) and the four
block DMAs are explicitly serialized so block 3 lands at full wire
bandwidth instead of sharing it 3 ways.
"""

import os

import numpy as np
import ml_dtypes

import concourse.bass as bass
import concourse.tile as tile
from concourse import bacc, mybir
from concourse import bass_utils
B, T, E, H = 8, 2048, 1024, 64
P = 128                       # partitions
CB = 512                      # column block (t) width
NE = E // P                   # 8 contraction chunks for projections
NJ = T // CB                  # 4 column blocks
SCALE = float(E) ** -0.5      # note: embed**-0.5, not head_size**-0.5
MASK_NEG = -1.0e30
F32 = mybir.dt.float32
BF16 = mybir.dt.bfloat16
X = mybir.AxisListType.X
EXP = mybir.ActivationFunctionType.Exp
COPY = mybir.ActivationFunctionType.Copy

# packed weights tensor column offsets (all bf16)
WKV0 = 0                      # [P, NE*P]   chunk e: [Wk_e | Wv_e]
WQ0 = WKV0 + NE * P           # [P, NE*H]
MASK0 = WQ0 + NE * H          # [P, 4*P]    additive -1e30 triangles
ID0 = MASK0 + 4 * P           # [P, P]      identity
WALLW = ID0 + P


def _emit(tc, xb_d, wall_d, out_d):
    nc = tc.nc
    from contextlib import ExitStack

    with ExitStack() as ctx:
        singles = ctx.enter_context(tc.tile_pool(name="singles", bufs=1))
        epool = ctx.enter_context(tc.tile_pool(name="erow", bufs=9))
        dpool = ctx.enter_context(tc.tile_pool(name="dsmall", bufs=12))
        vpool = ctx.enter_context(tc.tile_pool(name="vrow", bufs=9))
        ps = ctx.enter_context(tc.tile_pool(name="ps", bufs=2, space="PSUM"))
        pproj_pool = ctx.enter_context(
            tc.tile_pool(name="pproj", bufs=1, space="PSUM")
        )
        pout = ctx.enter_context(tc.tile_pool(name="pout", bufs=1, space="PSUM"))

        # weights: k/q/v gate the first projections, so they load first;
        # masks+identity live in a separate tile fetched a bit later so
        # the x fence graph stays acyclic
        wall = singles.tile([P, MASK0], BF16, name="wall")
        wall2 = singles.tile([P, WALLW - MASK0], BF16, name="wall2")
        wkv = wall[:, WKV0 : WKV0 + NE * P]
        wq = wall[:, WQ0 : WQ0 + NE * H]
        masks = wall2[:, 0 : 4 * P]
        identb = wall2[:, 4 * P : 5 * P]

        # x blocks: one dma_start tops out at ~165 GB/s, so each block is
        # split in half across two queues for two concurrent wire streams;
        # later blocks are paced behind earlier ones by tiny WAW fences on
        # the gpsimd queue (the fence reads block j+1, so it waits for that
        # transfer, and writes a 2-element span straddling both halves of
        # block j's tile, so both of block j's half-DMAs wait on it)
        xts = {
            j: singles.tile([P, NE * CB], BF16, name=f"x{j}")
            for j in range(NJ)
        }
        junk = singles.tile([P, CB], BF16)
        nc.gpsimd.memset(junk[:], 1.0)
        HALF = NE * CB // 2
        nc.scalar.dma_start(out=wall[:], in_=wall_d[:, 0:MASK0])
        nc.scalar.dma_start(out=xts[3][:, HALF:], in_=xb_d[3][:, HALF:])
        nc.scalar.dma_start(out=wall2[:], in_=wall_d[:, MASK0:])
        nc.sync.dma_start(out=xts[3][:, 0:HALF], in_=xb_d[3][:, 0:HALF])
        for j in (2, 1, 0):
            nc.gpsimd.tensor_copy(
                xts[j][0:1, HALF - 1 : HALF + 1], xts[j + 1][0:1, 0:2]
            )
            nc.sync.dma_start(out=xts[j][:, 0:HALF], in_=xb_d[j][:, 0:HALF])
            nc.gpsimd.dma_start(out=xts[j][:, HALF:], in_=xb_d[j][:, HALF:])

        # ~4us of dummy matmuls while the first DMAs land: primes the PE
        # activity monitor so the real chains start at 2.4 GHz, not 1.2
        pwarm = ps.tile([P, 2 * CB], F32, tag="ps", name="pwarm")
        for w in range(10):
            nc.tensor.matmul(
                pwarm[:, 0:CB],
                lhsT=junk[:, 0:P],
                rhs=junk[:],
                start=(w == 0),
                stop=(w == 9),
            )

        # per-block projected activations: [kT(0:64); vT(64:128)] cols 0:CB,
        # [qT(0:64); junk] cols CB:2CB (k and q share partition base 0 so
        # the S matmul's lhsT/rhs bases match)
        kqv = {
            j: singles.tile([P, 2 * CB], BF16, name=f"kqv{j}")
            for j in range(NJ)
        }

        # out^T accumulators packed 2 per bank: jj even rows 0:64, odd 64:128.
        # Accumulation groups on disjoint partition ranges of one bank are
        # fine on HW (per-element has_written); skip the sim's coarse check.
        pout_tiles = [
            pout.tile([P, CB], F32, tag=f"pt{a}", name=f"pt{a}") for a in range(2)
        ]
        outst = singles.tile([P, 2 * CB], F32, name="outst")

        def pout_slice(jj, c0, c1):
            rb = H * (jj % 2)
            return pout_tiles[jj // 2][rb : rb + H, c0:c1]

        # deferred AV emission (lag behind S so PE never waits on the
        # d / v' chain): each entry = (r, j_of_row), d0, erow, vi
        pending_av = []

        def _av_one(rj, d0, erow, vi, jj):
            c = (jj - rj[1]) * CB
            lo = d0 if jj == rj[1] else 0
            nc.tensor.matmul(
                pout_slice(jj, lo, CB),
                lhsT=vi[:],
                rhs=erow[:, c + lo : c + CB],
                start=(jj == rj[1] and rj[0] == 0),
                stop=(rj[1] == 0 and rj[0] == 3),
                skip_group_check=True,
            )

        def close_bank(a):
            # stage out^T bank a PSUM->SBUF (vector for bank 0, scalar for
            # bank 1 so the two copies overlap), then store it
            half = outst[:, a * CB : (a + 1) * CB]
            if a == 0:
                nc.vector.tensor_copy(half, pout_tiles[0][:])
            else:
                nc.scalar.activation(out=half, in_=pout_tiles[1][:], func=COPY)
            nc.sync.dma_start(
                out=out_d[:, a * CB : (a + 1) * CB], in_=half
            )

        def flush_av(final):
            if final:
                # group by output bank so bank A closes early and its copy
                # and store overlap bank B's last matmuls
                rows = list(pending_av)
                pending_av.clear()
                for jj in range(NJ):
                    for rj, d0, erow, vi in rows:
                        if jj >= rj[1]:
                            _av_one(rj, d0, erow, vi, jj)
                    if jj % 2 == 1:
                        close_bank(jj // 2)
                return
            rj, d0, erow, vi = pending_av.pop(0)
            for jj in range(rj[1], NJ):
                _av_one(rj, d0, erow, vi, jj)

        # projection matmul emission is spread through the PREVIOUS step's
        # rows so the PE instruction stream stays dense (HAM stays warm)
        def proj_thunks(j):
            pproj = pproj_pool.tile([P, 2 * CB], F32, tag="pp", name="pproj")
            thunks = []
            for e in range(NE):
                thunks.append(
                    lambda e=e: nc.tensor.matmul(
                        pproj[:, 0:CB],
                        lhsT=wkv[:, e * P : (e + 1) * P],
                        rhs=xts[j][:, e * CB : (e + 1) * CB],
                        start=(e == 0),
                        stop=(e == NE - 1),
                    )
                )
            for e in range(NE):
                thunks.append(
                    lambda e=e: nc.tensor.matmul(
                        pproj[0:H, CB : 2 * CB],
                        lhsT=wq[:, e * H : (e + 1) * H],
                        rhs=xts[j][:, e * CB : (e + 1) * CB],
                        start=(e == 0),
                        stop=(e == NE - 1),
                    )
                )
            return pproj, thunks

        def proj_cast(j, pproj):
            # one wide PSUM->SBUF bf16 cast for kT, qT and vT together
            # (vector, so the scalar engine stays dedicated to the exps)
            nc.vector.tensor_copy(kqv[j][:], pproj[:])

        # --- main pipeline: column blocks in descending order --------------
        next_proj = []  # pending matmul thunks for step j-1's projections

        def drip_proj(k):
            for _ in range(min(k, len(next_proj))):
                next_proj.pop(0)()

        pproj, thunks = proj_thunks(3)
        for t in thunks:
            t()
        proj_cast(3, pproj)

        for j in reversed(range(NJ)):
            if j > 0:
                pproj_next, next_proj = proj_thunks(j - 1)

            # rows i = 4j .. 4j+3 of E^T are now computable in full
            for r in range(4):
                i = 4 * j + r
                d0 = r * P  # local offset of this s-chunk within block j
                nblk = NJ - j
                npair = (nblk + 1) // 2
                erow = epool.tile([P, T], BF16)
                dparts = dpool.tile([P, 2], F32, tag="dparts")
                kT_sl = kqv[j][0:H, d0 : d0 + P]

                psts = []
                for pair in range(npair):
                    jj0 = j + 2 * pair
                    w = CB * min(2, NJ - jj0)  # 512 or 1024
                    if j == 0 and pair == 1:
                        # projection pool is idle during the last step
                        pst = pproj_pool.tile(
                            [P, 2 * CB], F32, tag="pp", name="pst0"
                        )
                    else:
                        pst = ps.tile([P, 2 * CB], F32, tag="ps")
                    psts.append((pst, jj0, w))

                # additive -1e30 triangle into PSUM via the PE (identity
                # stationary), then all S matmuls back-to-back with the
                # same kT stationary (single weight load with ldw-opt)
                pst0 = psts[0][0]
                nc.tensor.matmul(
                    pst0[:, d0 : d0 + P],
                    lhsT=identb,
                    rhs=masks[:, r * P : (r + 1) * P],
                    start=True,
                    stop=False,
                )
                for pst, jj0, w in psts:
                    for u in range(w // CB):
                        jj = jj0 + u
                        c0 = u * CB
                        if jj == j:
                            # diagonal block: accumulate onto the mask,
                            # then the clean remainder of the block
                            nc.tensor.matmul(
                                pst[:, d0 : d0 + P],
                                lhsT=kT_sl,
                                rhs=kqv[jj][0:H, CB + d0 : CB + d0 + P],
                                start=False,
                                stop=True,
                            )
                            if d0 + P < CB:
                                nc.tensor.matmul(
                                    pst[:, d0 + P : CB],
                                    lhsT=kT_sl,
                                    rhs=kqv[jj][0:H, CB + d0 + P : 2 * CB],
                                    start=True,
                                    stop=True,
                                )
                        else:
                            nc.tensor.matmul(
                                pst[:, c0 : c0 + CB],
                                lhsT=kT_sl,
                                rhs=kqv[jj][0:H, CB : 2 * CB],
                                start=True,
                                stop=True,
                            )
                drip_proj(3)

                # exp (+ d partial sums fused via accum_out), per pair
                for pair, (pst, jj0, w) in enumerate(psts):
                    lo = d0 if pair == 0 else 0
                    c = 2 * CB * pair
                    nc.scalar.activation(
                        out=erow[:, c + lo : c + w],
                        in_=pst[:, lo:w],
                        func=EXP,
                        scale=SCALE,
                        accum_out=dparts[:, pair : pair + 1],
                    )

                # d = sum over the row; 1/d feeds the v' scale
                dinv = dpool.tile([P, 1], F32, tag="dinv")
                if npair > 1:
                    dsum = dpool.tile([P, 1], F32, tag="dsum")
                    nc.vector.reduce_sum(dsum[:], dparts[:, 0:npair], axis=X)
                    nc.vector.reciprocal(dinv[:], dsum[:])
                else:
                    nc.vector.reciprocal(dinv[:], dparts[:, 0:1])

                pvt = ps.tile([P, 2 * CB], BF16, tag="ps", name="pvt")
                nc.tensor.transpose(
                    pvt[:, 0:H],
                    kqv[j][H:P, d0 : d0 + P],
                    identb[H:P, H:P],
                )
                vi = vpool.tile([P, H], BF16, tag="vi", name="vi")
                nc.vector.tensor_scalar_mul(vi[:], pvt[:, 0:H], dinv[:])

                lag = 5 if j == 1 else (2 if j == 0 else 2)
                while len(pending_av) >= lag:
                    flush_av(False)  # AV matmuls lag behind S for overlap
                drip_proj(2)
                pending_av.append(((r, j), d0, erow, vi))

            # drain remaining next-step projection matmuls, then its cast
            drip_proj(len(next_proj))
            if j > 0:
                proj_cast(j - 1, pproj_next)

        flush_av(True)


def _enable_ldw_opt():
    """Flip walrus's --enable-ldw-opt to true for our compile: consecutive
    matmuls reusing the same stationary operand then skip the reload."""
    import concourse.bass_utils as bu

    if getattr(bu, "_ldw_opt_patched", False):
        return
    orig = bu.run_command

    def run_command_ldw(cmd, *a, **kw):
        if isinstance(cmd, list):
            cmd = [
                "--enable-ldw-opt=true" if c == "--enable-ldw-opt=false" else c
                for c in cmd
            ]
        return orig(cmd, *a, **kw)

    bu.run_command = run_command_ldw
    bu._ldw_opt_patched = True


def _build_program():
    # NOTE: walrus rejects --enable-ldw-opt=true when the program contains
    # transpose-mode LDWEIGHTS (our pvt transposes), so this stays opt-in.
    if os.environ.get("BASS_LDW_OPT", "0") == "1":
        _enable_ldw_opt()
    nc = bacc.Bacc("TRN2", target_bir_lowering=False, debug=False, num_devices=B)
    xb_d = nc.dram_tensor("xb", [NJ, P, NE * CB], BF16, kind="ExternalInput").ap()
    wall_d = nc.dram_tensor("wall", [P, WALLW], BF16, kind="ExternalInput").ap()
    out_d = nc.dram_tensor("out", [P, 2 * CB], F32, kind="ExternalOutput").ap()
    with tile.TileContext(nc) as tc:
        _emit(tc, xb_d, wall_d, out_d)
    nc.compile()
    return nc


def _host_masks():
    """[128, 4*128] additive triangles: row r masks t < s within the
    diagonal 128-block (t-local f, partition p: keep f >= p)."""
    m = np.full((P, 4 * P), MASK_NEG, dtype=np.float32)
    p = np.arange(P)[:, None]
    f = np.arange(P)[None, :]
    for r in range(4):
        m[:, r * P : (r + 1) * P][f >= p] = 0.0
    return m


def _host_inputs(x, Wk, Wq, Wv):
    bf = ml_dtypes.bfloat16
    x = np.asarray(x, dtype=np.float32)
    # [B, E, T] -> block-major [B, NJ, P, NE*CB] so each block is one
    # contiguous DMA with 4KB/partition runs
    xT = np.transpose(x, (0, 2, 1)).reshape(B, NE, P, NJ, CB)
    xb = np.ascontiguousarray(xT.transpose(0, 3, 2, 1, 4)).reshape(
        B, NJ, P, NE * CB
    ).astype(bf)

    def chunks(w):  # [E, h] -> [NE, P, h]
        return np.asarray(w, np.float32).reshape(NE, P, -1)

    # wkv chunk e = [Wk_e | Wv_e] -> [P, NE*128]
    kv = np.concatenate([chunks(Wk), chunks(Wv)], axis=2)
    wkv = kv.transpose(1, 0, 2).reshape(P, NE * P)
    wqp = chunks(Wq).transpose(1, 0, 2).reshape(P, NE * H)
    wall = np.concatenate(
        [wkv, wqp, _host_masks(), np.eye(P, dtype=np.float32)], axis=1
    ).astype(bf)
    assert wall.shape == (P, WALLW)
    return [{"xb": xb[b], "wall": wall} for b in range(B)]


def _unpack_out(outT):
    """[128, 1024] out^T banks -> [T, H] natural layout."""
    o = np.empty((T, H), dtype=np.float32)
    for a in range(2):
        for h2 in range(2):
            jj = 2 * a + h2
            o[jj * CB : (jj + 1) * CB, :] = outT[
                H * h2 : H * (h2 + 1), a * CB : (a + 1) * CB
            ].T
    return o


def _ensure_axon_ntff_hook():
    """The agent image's antenv lacks axon_hooks; synthesize it so
    run_bass_kernel_spmd's trace path can find the NTFF profile hook."""
    import sys
    import types

    if "antenv.axon_hooks" in sys.modules:
        return
    try:
        import antenv

        mod = types.ModuleType("antenv.axon_hooks")
        mod._hook = None

        def set_axon_ntff_profile_hook(h):
            mod._hook = h

        def get_axon_ntff_profile_hook():
            return mod._hook

        mod.set_axon_ntff_profile_hook = set_axon_ntff_profile_hook
        mod.get_axon_ntff_profile_hook = get_axon_ntff_profile_hook
        sys.modules["antenv.axon_hooks"] = mod
        antenv.axon_hooks = mod

        from trn_agent_boot.trn_boot import _ntff_profile_via_ctypes

        hook = _ntff_profile_via_ctypes("/opt/axon/libaxon_pjrt.so")
        if hook is not None:
            mod._hook = hook
    except Exception as e:  # degrade to untraced run
        print(f"NTFF hook setup failed ({e}); tracing will be skipped")


def kernel(x, Wk, Wq, Wv, _trace=False, _trace_kwargs=None):
    if _trace:
        _ensure_axon_ntff_hook()
    in_maps = _host_inputs(x, Wk, Wq, Wv)
    nc = _build_program()
    res = bass_utils.run_bass_kernel_spmd(
        nc, in_maps, list(range(B)), trace=_trace, **(_trace_kwargs or {})
    )
    out = np.stack(
        [_unpack_out(res.results[b]["out"]) for b in range(B)], axis=0
    )
    if _trace:
        kernel.last_results = res
    return out.astype(np.float32)


# revision 22
# speedup vs baseline: 1.2954x; 1.0010x over previous
"""Trainium2 Bass kernel for a single attention head with query-axis softmax.

Reference semantics (per batch b):
    k = x @ Wk; q = x @ Wq; v = x @ Wv                 # [T, H]
    wei = (q @ k^T) * E**-0.5                          # [T(query), T(key)]
    wei = where(tril, wei, -inf)                       # causal: keep s <= t
    p = softmax(wei, axis=0 over query t)              # NOTE: query axis!
    out = p @ v                                        # [T, H]

Because the softmax normalizes over the query axis t (per key column s),
out[t,h] = sum_s E[t,s] * v[s,h] / d[s] with E[t,s] = exp(wei[t,s])
(zero for s > t) and d[s] = sum_t E[t,s].  The kernel computes E^T tiles
([s on partitions, t free]) so d is a free-axis row sum (fused into the
exp instruction via accum_out), scales v rows by 1/d, and accumulates
out^T on PE.  out^T is stored as-is; the host un-transposes during the
gather (free), so no on-device layout fixup is needed.

The causal triangle mask on the diagonal block is applied ON the PE:
a 128-row matmul against the identity writes the additive -1e30 triangle
into PSUM (start of the accumulation group), and the diagonal S matmul
accumulates on top of it - no vector/gpsimd op, no cross-engine hop.

Projections: k and q are packed into one 128-partition stationary
([kT; qT] stacked), v separate; one wide PSUM->SBUF cast per column
block moves all three.

Sharding: batch dim (8) across the 8 NeuronCores, weights replicated.
x is host-packed per column block ([NJ, 128, NE*CB] bf16) and the four
block DMAs are explicitly serialized so block 3 lands at full wire
bandwidth instead of sharing it 3 ways.
"""

import os

import numpy as np
import ml_dtypes

import concourse.bass as bass
import concourse.tile as tile
from concourse import bacc, mybir
from concourse import bass_utils
B, T, E, H = 8, 2048, 1024, 64
P = 128                       # partitions
CB = 512                      # column block (t) width
NE = E // P                   # 8 contraction chunks for projections
NJ = T // CB                  # 4 column blocks
SCALE = float(E) ** -0.5      # note: embed**-0.5, not head_size**-0.5
MASK_NEG = -1.0e30
F32 = mybir.dt.float32
BF16 = mybir.dt.bfloat16
X = mybir.AxisListType.X
EXP = mybir.ActivationFunctionType.Exp
COPY = mybir.ActivationFunctionType.Copy

# packed weights tensor column offsets (all bf16)
WKV0 = 0                      # [P, NE*P]   chunk e: [Wk_e | Wv_e]
WQ0 = WKV0 + NE * P           # [P, NE*H]
MASK0 = WQ0 + NE * H          # [P, 4*P]    additive -1e30 triangles
ID0 = MASK0 + 4 * P           # [P, P]      identity
WALLW = ID0 + P


def _emit(tc, xb_d, wall_d, out_d):
    nc = tc.nc
    from contextlib import ExitStack

    with ExitStack() as ctx:
        singles = ctx.enter_context(tc.tile_pool(name="singles", bufs=1))
        epool = ctx.enter_context(tc.tile_pool(name="erow", bufs=9))
        dpool = ctx.enter_context(tc.tile_pool(name="dsmall", bufs=12))
        vpool = ctx.enter_context(tc.tile_pool(name="vrow", bufs=9))
        vspool = ctx.enter_context(tc.tile_pool(name="vstage", bufs=2))
        ps = ctx.enter_context(tc.tile_pool(name="ps", bufs=2, space="PSUM"))
        pproj_pool = ctx.enter_context(
            tc.tile_pool(name="pproj", bufs=1, space="PSUM")
        )
        pout = ctx.enter_context(tc.tile_pool(name="pout", bufs=1, space="PSUM"))

        # weights: k/q/v gate the first projections, so they load first;
        # masks+identity live in a separate tile fetched a bit later so
        # the x fence graph stays acyclic
        wall = singles.tile([P, MASK0], BF16, name="wall")
        wall2 = singles.tile([P, WALLW - MASK0], BF16, name="wall2")
        wkv = wall[:, WKV0 : WKV0 + NE * P]
        wq = wall[:, WQ0 : WQ0 + NE * H]
        masks = wall2[:, 0 : 4 * P]
        identb = wall2[:, 4 * P : 5 * P]

        # x blocks: one dma_start tops out at ~165 GB/s, so each block is
        # split in half across two queues for two concurrent wire streams;
        # later blocks are paced behind earlier ones by tiny WAW fences on
        # the gpsimd queue (the fence reads block j+1, so it waits for that
        # transfer, and writes a 2-element span straddling both halves of
        # block j's tile, so both of block j's half-DMAs wait on it)
        xts = {
            j: singles.tile([P, NE * CB], BF16, name=f"x{j}")
            for j in range(NJ)
        }
        junk = singles.tile([P, CB], BF16)
        nc.gpsimd.memset(junk[:], 1.0)
        HALF = NE * CB // 2
        # open with exactly two streams: weights (sync) + x3 second half
        # (scalar); x3 first half is WAW-fenced behind the weights
        nc.sync.dma_start(out=wall[:], in_=wall_d[:, 0:MASK0])
        nc.scalar.dma_start(out=xts[3][:, HALF:], in_=xb_d[3][:, HALF:])
        nc.scalar.dma_start(out=wall2[:], in_=wall_d[:, MASK0:])
        nc.gpsimd.tensor_copy(xts[3][0:1, 0:1], wall[0:1, 0:1])
        nc.sync.dma_start(out=xts[3][:, 0:HALF], in_=xb_d[3][:, 0:HALF])
        for j in (2, 1, 0):
            nc.gpsimd.tensor_copy(
                xts[j][0:1, HALF - 1 : HALF + 1], xts[j + 1][0:1, 0:2]
            )
            nc.sync.dma_start(out=xts[j][:, 0:HALF], in_=xb_d[j][:, 0:HALF])
            nc.gpsimd.dma_start(out=xts[j][:, HALF:], in_=xb_d[j][:, HALF:])

        # ~4us of dummy matmuls while the first DMAs land: primes the PE
        # activity monitor so the real chains start at 2.4 GHz, not 1.2
        pwarm = ps.tile([P, 2 * CB], F32, tag="ps", name="pwarm")
        for w in range(11):
            nc.tensor.matmul(
                pwarm[:, 0:CB],
                lhsT=junk[:, 0:P],
                rhs=junk[:],
                start=(w == 0),
                stop=(w == 10),
            )

        # per-block projected activations: [kT(0:64); vT(64:128)] cols 0:CB,
        # [qT(0:64); junk] cols CB:2CB (k and q share partition base 0 so
        # the S matmul's lhsT/rhs bases match)
        kqv = {
            j: singles.tile([P, 2 * CB], BF16, name=f"kqv{j}")
            for j in range(NJ)
        }

        # out^T accumulators packed 2 per bank: jj even rows 0:64, odd 64:128.
        # Accumulation groups on disjoint partition ranges of one bank are
        # fine on HW (per-element has_written); skip the sim's coarse check.
        pout_tiles = [
            pout.tile([P, CB], F32, tag=f"pt{a}", name=f"pt{a}") for a in range(2)
        ]
        outst = singles.tile([P, 2 * CB], F32, name="outst")

        def pout_slice(jj, c0, c1):
            rb = H * (jj % 2)
            return pout_tiles[jj // 2][rb : rb + H, c0:c1]

        # deferred AV emission (lag behind S so PE never waits on the
        # d / v' chain): each entry = (r, j_of_row), d0, erow, vi
        pending_av = []

        def _av_one(rj, d0, erow, vi, jj):
            c = (jj - rj[1]) * CB
            lo = d0 if jj == rj[1] else 0
            nc.tensor.matmul(
                pout_slice(jj, lo, CB),
                lhsT=vi[:],
                rhs=erow[:, c + lo : c + CB],
                start=(jj == rj[1] and rj[0] == 0),
                stop=(rj[1] == 0 and rj[0] == 3),
                skip_group_check=True,
            )

        def close_bank(a):
            # stage out^T bank a PSUM->SBUF (vector for bank 0, scalar for
            # bank 1 so the two copies overlap), then store it
            half = outst[:, a * CB : (a + 1) * CB]
            if a == 0:
                nc.vector.tensor_copy(half, pout_tiles[0][:])
            else:
                nc.scalar.activation(out=half, in_=pout_tiles[1][:], func=COPY)
            nc.sync.dma_start(
                out=out_d[:, a * CB : (a + 1) * CB], in_=half
            )

        def flush_av(final):
            if final:
                # group by output bank so bank A closes early and its copy
                # and store overlap bank B's last matmuls
                rows = list(pending_av)
                pending_av.clear()
                for jj in range(NJ):
                    for rj, d0, erow, vi in rows:
                        if jj >= rj[1]:
                            _av_one(rj, d0, erow, vi, jj)
                    if jj % 2 == 1:
                        close_bank(jj // 2)
                return
            rj, d0, erow, vi = pending_av.pop(0)
            for jj in range(rj[1], NJ):
                _av_one(rj, d0, erow, vi, jj)

        # projection matmul emission is spread through the PREVIOUS step's
        # rows so the PE instruction stream stays dense (HAM stays warm)
        def proj_thunks(j):
            pproj = pproj_pool.tile([P, 2 * CB], F32, tag="pp", name="pproj")
            thunks = []
            for e in range(NE):
                thunks.append(
                    lambda e=e: nc.tensor.matmul(
                        pproj[:, 0:CB],
                        lhsT=wkv[:, e * P : (e + 1) * P],
                        rhs=xts[j][:, e * CB : (e + 1) * CB],
                        start=(e == 0),
                        stop=(e == NE - 1),
                    )
                )
            for e in range(NE):
                thunks.append(
                    lambda e=e: nc.tensor.matmul(
                        pproj[0:H, CB : 2 * CB],
                        lhsT=wq[:, e * H : (e + 1) * H],
                        rhs=xts[j][:, e * CB : (e + 1) * CB],
                        start=(e == 0),
                        stop=(e == NE - 1),
                    )
                )
            return pproj, thunks

        def proj_cast(j, pproj):
            # one wide PSUM->SBUF bf16 cast for kT, qT and vT together
            # (vector, so the scalar engine stays dedicated to the exps)
            nc.vector.tensor_copy(kqv[j][:], pproj[:])

        # --- main pipeline: column blocks in descending order --------------
        next_proj = []  # pending matmul thunks for step j-1's projections

        def drip_proj(k):
            for _ in range(min(k, len(next_proj))):
                next_proj.pop(0)()

        pproj, thunks = proj_thunks(3)
        for t in thunks:
            t()
        proj_cast(3, pproj)

        for j in reversed(range(NJ)):
            if j > 0:
                pproj_next, next_proj = proj_thunks(j - 1)

            # batch-transpose the step's four v chunks into one PSUM tile
            # and stage them in SBUF unscaled: the per-row v' scale then
            # becomes a cheap SBUF-only multiply, and the S-pair PSUM ring
            # no longer threads through vector's per-row work
            vps = ps.tile([P, 2 * CB], BF16, tag="ps", name="vps")
            for r in range(4):
                nc.tensor.transpose(
                    vps[:, r * H : (r + 1) * H],
                    kqv[j][H:P, r * P : (r + 1) * P],
                    identb[H:P, H:P],
                )
            vstage = vspool.tile([P, 4 * H], BF16, name="vstage")
            nc.vector.tensor_copy(vstage[:], vps[:, 0 : 4 * H])
            drip_proj(3)

            # rows i = 4j .. 4j+3 of E^T are now computable in full
            for r in range(4):
                i = 4 * j + r
                d0 = r * P  # local offset of this s-chunk within block j
                nblk = NJ - j
                npair = (nblk + 1) // 2
                erow = epool.tile([P, T], BF16)
                dparts = dpool.tile([P, 2], F32, tag="dparts")
                kT_sl = kqv[j][0:H, d0 : d0 + P]

                psts = []
                for pair in range(npair):
                    jj0 = j + 2 * pair
                    w = CB * min(2, NJ - jj0)  # 512 or 1024
                    if j == 0 and pair == 1:
                        # projection pool is idle during the last step
                        pst = pproj_pool.tile(
                            [P, 2 * CB], F32, tag="pp", name="pst0"
                        )
                    else:
                        pst = ps.tile([P, 2 * CB], F32, tag="ps")
                    psts.append((pst, jj0, w))

                # additive -1e30 triangle into PSUM via the PE (identity
                # stationary), then all S matmuls back-to-back with the
                # same kT stationary (single weight load with ldw-opt)
                pst0 = psts[0][0]
                nc.tensor.matmul(
                    pst0[:, d0 : d0 + P],
                    lhsT=identb,
                    rhs=masks[:, r * P : (r + 1) * P],
                    start=True,
                    stop=False,
                )
                for pair, (pst, jj0, w) in enumerate(psts):
                    if j == 0 and pair == 1 and pending_av:
                        # the borrowed pair-1 tile serializes on the prior
                        # row's exp; keep ready AV work ahead of it in the
                        # queue so the PE never drains
                        flush_av(False)
                    for u in range(w // CB):
                        jj = jj0 + u
                        c0 = u * CB
                        if jj == j:
                            # diagonal block: accumulate onto the mask,
                            # then the clean remainder of the block
                            nc.tensor.matmul(
                                pst[:, d0 : d0 + P],
                                lhsT=kT_sl,
                                rhs=kqv[jj][0:H, CB + d0 : CB + d0 + P],
                                start=False,
                                stop=True,
                            )
                            if d0 + P < CB:
                                nc.tensor.matmul(
                                    pst[:, d0 + P : CB],
                                    lhsT=kT_sl,
                                    rhs=kqv[jj][0:H, CB + d0 + P : 2 * CB],
                                    start=True,
                                    stop=True,
                                )
                        else:
                            nc.tensor.matmul(
                                pst[:, c0 : c0 + CB],
                                lhsT=kT_sl,
                                rhs=kqv[jj][0:H, CB : 2 * CB],
                                start=True,
                                stop=True,
                            )
                drip_proj(3)

                # exp (+ d partial sums fused via accum_out), per pair
                for pair, (pst, jj0, w) in enumerate(psts):
                    lo = d0 if pair == 0 else 0
                    c = 2 * CB * pair
                    nc.scalar.activation(
                        out=erow[:, c + lo : c + w],
                        in_=pst[:, lo:w],
                        func=EXP,
                        scale=SCALE,
                        accum_out=dparts[:, pair : pair + 1],
                    )

                # d = sum over the row; 1/d feeds the v' scale
                dinv = dpool.tile([P, 1], F32, tag="dinv")
                if npair > 1:
                    dsum = dpool.tile([P, 1], F32, tag="dsum")
                    nc.vector.reduce_sum(dsum[:], dparts[:, 0:npair], axis=X)
                    nc.vector.reciprocal(dinv[:], dsum[:])
                else:
                    nc.vector.reciprocal(dinv[:], dparts[:, 0:1])

                vi = vpool.tile([P, H], BF16, tag="vi", name="vi")
                nc.vector.tensor_scalar_mul(
                    vi[:], vstage[:, r * H : (r + 1) * H], dinv[:]
                )

                lag = 5 if j == 1 else (2 if j == 0 else 2)
                while len(pending_av) >= lag:
                    flush_av(False)  # AV matmuls lag behind S for overlap
                drip_proj(2)
                pending_av.append(((r, j), d0, erow, vi))

            # drain remaining next-step projection matmuls, then its cast
            drip_proj(len(next_proj))
            if j > 0:
                proj_cast(j - 1, pproj_next)

        flush_av(True)


def _enable_ldw_opt():
    """Flip walrus's --enable-ldw-opt to true for our compile: consecutive
    matmuls reusing the same stationary operand then skip the reload."""
    import concourse.bass_utils as bu

    if getattr(bu, "_ldw_opt_patched", False):
        return
    orig = bu.run_command

    def run_command_ldw(cmd, *a, **kw):
        if isinstance(cmd, list):
            cmd = [
                "--enable-ldw-opt=true" if c == "--enable-ldw-opt=false" else c
                for c in cmd
            ]
        return orig(cmd, *a, **kw)

    bu.run_command = run_command_ldw
    bu._ldw_opt_patched = True


def _build_program():
    # NOTE: walrus rejects --enable-ldw-opt=true when the program contains
    # transpose-mode LDWEIGHTS (our pvt transposes), so this stays opt-in.
    if os.environ.get("BASS_LDW_OPT", "0") == "1":
        _enable_ldw_opt()
    nc = bacc.Bacc("TRN2", target_bir_lowering=False, debug=False, num_devices=B)
    xb_d = nc.dram_tensor("xb", [NJ, P, NE * CB], BF16, kind="ExternalInput").ap()
    wall_d = nc.dram_tensor("wall", [P, WALLW], BF16, kind="ExternalInput").ap()
    out_d = nc.dram_tensor("out", [P, 2 * CB], F32, kind="ExternalOutput").ap()
    with tile.TileContext(nc) as tc:
        _emit(tc, xb_d, wall_d, out_d)
    nc.compile()
    return nc


def _host_masks():
    """[128, 4*128] additive triangles: row r masks t < s within the
    diagonal 128-block (t-local f, partition p: keep f >= p)."""
    m = np.full((P, 4 * P), MASK_NEG, dtype=np.float32)
    p = np.arange(P)[:, None]
    f = np.arange(P)[None, :]
    for r in range(4):
        m[:, r * P : (r + 1) * P][f >= p] = 0.0
    return m


def _host_inputs(x, Wk, Wq, Wv):
    bf = ml_dtypes.bfloat16
    x = np.asarray(x, dtype=np.float32)
    # [B, E, T] -> block-major [B, NJ, P, NE*CB] so each block is one
    # contiguous DMA with 4KB/partition runs
    xT = np.transpose(x, (0, 2, 1)).reshape(B, NE, P, NJ, CB)
    xb = np.ascontiguousarray(xT.transpose(0, 3, 2, 1, 4)).reshape(
        B, NJ, P, NE * CB
    ).astype(bf)

    def chunks(w):  # [E, h] -> [NE, P, h]
        return np.asarray(w, np.float32).reshape(NE, P, -1)

    # wkv chunk e = [Wk_e | Wv_e] -> [P, NE*128]
    kv = np.concatenate([chunks(Wk), chunks(Wv)], axis=2)
    wkv = kv.transpose(1, 0, 2).reshape(P, NE * P)
    wqp = chunks(Wq).transpose(1, 0, 2).reshape(P, NE * H)
    wall = np.concatenate(
        [wkv, wqp, _host_masks(), np.eye(P, dtype=np.float32)], axis=1
    ).astype(bf)
    assert wall.shape == (P, WALLW)
    return [{"xb": xb[b], "wall": wall} for b in range(B)]


def _unpack_out(outT):
    """[128, 1024] out^T banks -> [T, H] natural layout."""
    o = np.empty((T, H), dtype=np.float32)
    for a in range(2):
        for h2 in range(2):
            jj = 2 * a + h2
            o[jj * CB : (jj + 1) * CB, :] = outT[
                H * h2 : H * (h2 + 1), a * CB : (a + 1) * CB
            ].T
    return o


def _ensure_axon_ntff_hook():
    """The agent image's antenv lacks axon_hooks; synthesize it so
    run_bass_kernel_spmd's trace path can find the NTFF profile hook."""
    import sys
    import types

    if "antenv.axon_hooks" in sys.modules:
        return
    try:
        import antenv

        mod = types.ModuleType("antenv.axon_hooks")
        mod._hook = None

        def set_axon_ntff_profile_hook(h):
            mod._hook = h

        def get_axon_ntff_profile_hook():
            return mod._hook

        mod.set_axon_ntff_profile_hook = set_axon_ntff_profile_hook
        mod.get_axon_ntff_profile_hook = get_axon_ntff_profile_hook
        sys.modules["antenv.axon_hooks"] = mod
        antenv.axon_hooks = mod

        from trn_agent_boot.trn_boot import _ntff_profile_via_ctypes

        hook = _ntff_profile_via_ctypes("/opt/axon/libaxon_pjrt.so")
        if hook is not None:
            mod._hook = hook
    except Exception as e:  # degrade to untraced run
        print(f"NTFF hook setup failed ({e}); tracing will be skipped")


def kernel(x, Wk, Wq, Wv, _trace=False, _trace_kwargs=None):
    if _trace:
        _ensure_axon_ntff_hook()
    in_maps = _host_inputs(x, Wk, Wq, Wv)
    nc = _build_program()
    res = bass_utils.run_bass_kernel_spmd(
        nc, in_maps, list(range(B)), trace=_trace, **(_trace_kwargs or {})
    )
    out = np.stack(
        [_unpack_out(res.results[b]["out"]) for b in range(B)], axis=0
    )
    if _trace:
        kernel.last_results = res
    return out.astype(np.float32)


# revision 26
# speedup vs baseline: 1.3319x; 1.0282x over previous
"""Trainium2 Bass kernel for a single attention head with query-axis softmax.

Reference semantics (per batch b):
    k = x @ Wk; q = x @ Wq; v = x @ Wv                 # [T, H]
    wei = (q @ k^T) * E**-0.5                          # [T(query), T(key)]
    wei = where(tril, wei, -inf)                       # causal: keep s <= t
    p = softmax(wei, axis=0 over query t)              # NOTE: query axis!
    out = p @ v                                        # [T, H]

Because the softmax normalizes over the query axis t (per key column s),
out[t,h] = sum_s E[t,s] * v[s,h] / d[s] with E[t,s] = exp(wei[t,s])
(zero for s > t) and d[s] = sum_t E[t,s].  The kernel computes E^T tiles
([s on partitions, t free]) so d is a free-axis row sum (fused into the
exp instruction via accum_out), scales v rows by 1/d, and accumulates
out^T on PE.  out^T is stored as-is; the host un-transposes during the
gather (free), so no on-device layout fixup is needed.

The causal triangle mask on the diagonal block is applied ON the PE:
a 128-row matmul against the identity writes the additive -1e30 triangle
into PSUM (start of the accumulation group), and the diagonal S matmul
accumulates on top of it - no vector/gpsimd op, no cross-engine hop.

Projections: k and q are packed into one 128-partition stationary
([kT; qT] stacked), v separate; one wide PSUM->SBUF cast per column
block moves all three.

Sharding: batch dim (8) across the 8 NeuronCores, weights replicated.
x is host-packed per column block ([NJ, 128, NE*CB] bf16) and the four
block DMAs are explicitly serialized so block 3 lands at full wire
bandwidth instead of sharing it 3 ways.
"""

import os

import numpy as np
import ml_dtypes

import concourse.bass as bass
import concourse.tile as tile
from concourse import bacc, mybir
from concourse import bass_utils
B, T, E, H = 8, 2048, 1024, 64
P = 128                       # partitions
CB = 512                      # column block (t) width
NE = E // P                   # 8 contraction chunks for projections
NJ = T // CB                  # 4 column blocks
SCALE = float(E) ** -0.5      # note: embed**-0.5, not head_size**-0.5
MASK_NEG = -1.0e30
F32 = mybir.dt.float32
BF16 = mybir.dt.bfloat16
X = mybir.AxisListType.X
EXP = mybir.ActivationFunctionType.Exp
COPY = mybir.ActivationFunctionType.Copy

# packed weights tensor column offsets (all bf16)
WKV0 = 0                      # [P, NE*P]   chunk e: [Wk_e | Wv_e]
WQ0 = WKV0 + NE * P           # [P, NE*H]
MASK0 = WQ0 + NE * H          # [P, 4*P]    additive -1e30 triangles
ID0 = MASK0 + 4 * P           # [P, P]      identity
WALLW = ID0 + P


def _emit(tc, xb_d, wall_d, out_d):
    nc = tc.nc
    from contextlib import ExitStack

    with ExitStack() as ctx:
        singles = ctx.enter_context(tc.tile_pool(name="singles", bufs=1))
        epool = ctx.enter_context(tc.tile_pool(name="erow", bufs=9))
        dpool = ctx.enter_context(tc.tile_pool(name="dsmall", bufs=12))
        vpool = ctx.enter_context(tc.tile_pool(name="vrow", bufs=9))
        vspool = ctx.enter_context(tc.tile_pool(name="vstage", bufs=2))
        ps = ctx.enter_context(tc.tile_pool(name="ps", bufs=2, space="PSUM"))
        pproj_pool = ctx.enter_context(
            tc.tile_pool(name="pproj", bufs=1, space="PSUM")
        )
        pout = ctx.enter_context(tc.tile_pool(name="pout", bufs=1, space="PSUM"))

        # weights: k/q/v gate the first projections, so they load first;
        # masks+identity live in a separate tile fetched a bit later so
        # the x fence graph stays acyclic
        wall = singles.tile([P, MASK0], BF16, name="wall")
        wall2 = singles.tile([P, WALLW - MASK0], BF16, name="wall2")
        wkv = wall[:, WKV0 : WKV0 + NE * P]
        wq = wall[:, WQ0 : WQ0 + NE * H]
        masks = wall2[:, 0 : 4 * P]
        identb = wall2[:, 4 * P : 5 * P]

        # x blocks: one dma_start tops out at ~165 GB/s and per-stream rate
        # degrades with concurrency, so each block is two separate tiles
        # (e-chunks 0-3 / 4-7) moved by two concurrent streams, and later
        # blocks are paced behind earlier ones with tiny WAW fences on the
        # gpsimd queue.  Separate tiles make the projection's data deps
        # piece-granular: the kv chain starts as soon as lo lands.
        HALF = NE * CB // 2
        xlo = {
            j: singles.tile([P, HALF], BF16, name=f"xlo{j}")
            for j in range(NJ)
        }
        xhi = {
            j: singles.tile([P, HALF], BF16, name=f"xhi{j}")
            for j in range(NJ)
        }
        junk = singles.tile([P, CB], BF16)
        nc.gpsimd.memset(junk[:], 1.0)
        # open with exactly two streams: weights (sync) + x3 lo (scalar);
        # x3 hi is WAW-fenced behind the weights
        nc.sync.dma_start(out=wall[:], in_=wall_d[:, 0:MASK0])
        nc.scalar.dma_start(out=xlo[3][:], in_=xb_d[3][:, 0:HALF])
        nc.scalar.dma_start(out=wall2[:], in_=wall_d[:, MASK0:])
        nc.gpsimd.tensor_copy(xhi[3][0:1, 0:1], wall[0:1, 0:1])
        nc.gpsimd.dma_start(out=xhi[3][:], in_=xb_d[3][:, HALF:])
        for j in (2, 1, 0):
            # cross-paired fences: block j's lo waits block j+1's hi and
            # vice versa, so block j starts only once block j+1 is done
            nc.gpsimd.tensor_copy(xlo[j][0:1, 0:1], xhi[j + 1][0:1, 0:1])
            nc.gpsimd.tensor_copy(xhi[j][0:1, 0:1], xlo[j + 1][0:1, 0:1])
            nc.sync.dma_start(out=xlo[j][:], in_=xb_d[j][:, 0:HALF])
            nc.gpsimd.dma_start(out=xhi[j][:], in_=xb_d[j][:, HALF:])

        # ~4us of dummy matmuls while the first DMAs land: primes the PE
        # activity monitor so the real chains start at 2.4 GHz, not 1.2
        pwarm = ps.tile([P, 2 * CB], F32, tag="ps", name="pwarm")
        for w in range(9):
            nc.tensor.matmul(
                pwarm[:, 0:CB],
                lhsT=junk[:, 0:P],
                rhs=junk[:],
                start=(w == 0),
                stop=(w == 8),
            )

        # per-block projected activations: [kT(0:64); vT(64:128)] cols 0:CB,
        # [qT(0:64); junk] cols CB:2CB (k and q share partition base 0 so
        # the S matmul's lhsT/rhs bases match)
        kqv = {
            j: singles.tile([P, 2 * CB], BF16, name=f"kqv{j}")
            for j in range(NJ)
        }

        # out^T accumulators packed 2 per bank: jj even rows 0:64, odd 64:128.
        # Accumulation groups on disjoint partition ranges of one bank are
        # fine on HW (per-element has_written); skip the sim's coarse check.
        pout_tiles = [
            pout.tile([P, CB], F32, tag=f"pt{a}", name=f"pt{a}") for a in range(2)
        ]
        outst = singles.tile([P, 2 * CB], F32, name="outst")

        def pout_slice(jj, c0, c1):
            rb = H * (jj % 2)
            return pout_tiles[jj // 2][rb : rb + H, c0:c1]

        # deferred AV emission (lag behind S so PE never waits on the
        # d / v' chain): each entry = (r, j_of_row), d0, erow, vi
        pending_av = []

        def _av_one(rj, d0, erow, vi, jj):
            c = (jj - rj[1]) * CB
            lo = d0 if jj == rj[1] else 0
            nc.tensor.matmul(
                pout_slice(jj, lo, CB),
                lhsT=vi[:],
                rhs=erow[:, c + lo : c + CB],
                start=(jj == rj[1] and rj[0] == 0),
                stop=(rj[1] == 0 and rj[0] == 3),
                skip_group_check=True,
            )

        def close_bank(a):
            # stage out^T bank a PSUM->SBUF (vector for bank 0, scalar for
            # bank 1 so the two copies overlap), then store it
            half = outst[:, a * CB : (a + 1) * CB]
            if a == 0:
                nc.vector.tensor_copy(half, pout_tiles[0][:])
            else:
                nc.scalar.activation(out=half, in_=pout_tiles[1][:], func=COPY)
            nc.sync.dma_start(
                out=out_d[:, a * CB : (a + 1) * CB], in_=half
            )

        def flush_av(final):
            if final:
                # group by output bank so bank A closes early and its copy
                # and store overlap bank B's last matmuls
                rows = list(pending_av)
                pending_av.clear()
                for jj in range(NJ):
                    for rj, d0, erow, vi in rows:
                        if jj >= rj[1]:
                            _av_one(rj, d0, erow, vi, jj)
                    if jj % 2 == 1:
                        close_bank(jj // 2)
                return
            rj, d0, erow, vi = pending_av.pop(0)
            for jj in range(rj[1], NJ):
                _av_one(rj, d0, erow, vi, jj)

        # projection matmul emission is spread through the PREVIOUS step's
        # rows so the PE instruction stream stays dense (HAM stays warm)
        def x_rhs(j, e):
            t_ = xlo[j] if e < NE // 2 else xhi[j]
            c = (e % (NE // 2)) * CB
            return t_[:, c : c + CB]

        def proj_thunks(j):
            pproj = pproj_pool.tile([P, 2 * CB], F32, tag="pp", name="pproj")
            thunks = []
            for e in range(NE):
                thunks.append(
                    lambda e=e: nc.tensor.matmul(
                        pproj[:, 0:CB],
                        lhsT=wkv[:, e * P : (e + 1) * P],
                        rhs=x_rhs(j, e),
                        start=(e == 0),
                        stop=(e == NE - 1),
                    )
                )
            for e in range(NE):
                thunks.append(
                    lambda e=e: nc.tensor.matmul(
                        pproj[0:H, CB : 2 * CB],
                        lhsT=wq[:, e * H : (e + 1) * H],
                        rhs=x_rhs(j, e),
                        start=(e == 0),
                        stop=(e == NE - 1),
                    )
                )
            return pproj, thunks

        def proj_cast(j, pproj):
            # one wide PSUM->SBUF bf16 cast for kT, qT and vT together
            # (vector, so the scalar engine stays dedicated to the exps)
            nc.vector.tensor_copy(kqv[j][:], pproj[:])

        # --- main pipeline: column blocks in descending order --------------
        next_proj = []  # pending matmul thunks for step j-1's projections

        def drip_proj(k):
            for _ in range(min(k, len(next_proj))):
                next_proj.pop(0)()

        pproj, thunks = proj_thunks(3)
        for t in thunks:
            t()
        proj_cast(3, pproj)

        for j in reversed(range(NJ)):
            if j > 0:
                pproj_next, next_proj = proj_thunks(j - 1)

            # batch-transpose the step's four v chunks into one PSUM tile
            # and stage them in SBUF unscaled: the per-row v' scale then
            # becomes a cheap SBUF-only multiply, and the S-pair PSUM ring
            # no longer threads through vector's per-row work
            vps = ps.tile([P, 2 * CB], BF16, tag="ps", name="vps")
            for r in range(4):
                nc.tensor.transpose(
                    vps[:, r * H : (r + 1) * H],
                    kqv[j][H:P, r * P : (r + 1) * P],
                    identb[H:P, H:P],
                )
            vstage = vspool.tile([P, 4 * H], BF16, name="vstage")
            nc.vector.tensor_copy(vstage[:], vps[:, 0 : 4 * H])
            drip_proj(3)

            # rows i = 4j .. 4j+3 of E^T are now computable in full
            for r in range(4):
                i = 4 * j + r
                d0 = r * P  # local offset of this s-chunk within block j
                nblk = NJ - j
                npair = (nblk + 1) // 2
                erow = epool.tile([P, T], BF16)
                dparts = dpool.tile([P, 2], F32, tag="dparts")
                kT_sl = kqv[j][0:H, d0 : d0 + P]

                psts = []
                for pair in range(npair):
                    jj0 = j + 2 * pair
                    w = CB * min(2, NJ - jj0)  # 512 or 1024
                    if j == 0 and pair == 1:
                        # projection pool is idle during the last step
                        pst = pproj_pool.tile(
                            [P, 2 * CB], F32, tag="pp", name="pst0"
                        )
                    else:
                        pst = ps.tile([P, 2 * CB], F32, tag="ps")
                    psts.append((pst, jj0, w))

                # additive -1e30 triangle into PSUM via the PE (identity
                # stationary), then all S matmuls back-to-back with the
                # same kT stationary (single weight load with ldw-opt)
                pst0 = psts[0][0]
                nc.tensor.matmul(
                    pst0[:, d0 : d0 + P],
                    lhsT=identb,
                    rhs=masks[:, r * P : (r + 1) * P],
                    start=True,
                    stop=False,
                )
                for pair, (pst, jj0, w) in enumerate(psts):
                    if j == 0 and pair == 1 and pending_av:
                        # the borrowed pair-1 tile serializes on the prior
                        # row's exp; keep ready AV work ahead of it in the
                        # queue so the PE never drains
                        flush_av(False)
                    for u in range(w // CB):
                        jj = jj0 + u
                        c0 = u * CB
                        if jj == j:
                            # diagonal block: accumulate onto the mask,
                            # then the clean remainder of the block
                            nc.tensor.matmul(
                                pst[:, d0 : d0 + P],
                                lhsT=kT_sl,
                                rhs=kqv[jj][0:H, CB + d0 : CB + d0 + P],
                                start=False,
                                stop=True,
                            )
                            if d0 + P < CB:
                                nc.tensor.matmul(
                                    pst[:, d0 + P : CB],
                                    lhsT=kT_sl,
                                    rhs=kqv[jj][0:H, CB + d0 + P : 2 * CB],
                                    start=True,
                                    stop=True,
                                )
                        else:
                            nc.tensor.matmul(
                                pst[:, c0 : c0 + CB],
                                lhsT=kT_sl,
                                rhs=kqv[jj][0:H, CB : 2 * CB],
                                start=True,
                                stop=True,
                            )
                drip_proj(3)

                # exp (+ d partial sums fused via accum_out), per pair
                for pair, (pst, jj0, w) in enumerate(psts):
                    lo = d0 if pair == 0 else 0
                    c = 2 * CB * pair
                    nc.scalar.activation(
                        out=erow[:, c + lo : c + w],
                        in_=pst[:, lo:w],
                        func=EXP,
                        scale=SCALE,
                        accum_out=dparts[:, pair : pair + 1],
                    )

                # d = sum over the row; 1/d feeds the v' scale
                dinv = dpool.tile([P, 1], F32, tag="dinv")
                if npair > 1:
                    dsum = dpool.tile([P, 1], F32, tag="dsum")
                    nc.vector.reduce_sum(dsum[:], dparts[:, 0:npair], axis=X)
                    nc.vector.reciprocal(dinv[:], dsum[:])
                else:
                    nc.vector.reciprocal(dinv[:], dparts[:, 0:1])

                vi = vpool.tile([P, H], BF16, tag="vi", name="vi")
                nc.vector.tensor_scalar_mul(
                    vi[:], vstage[:, r * H : (r + 1) * H], dinv[:]
                )

                lag = 5 if j == 1 else (2 if j == 0 else 2)
                while len(pending_av) >= lag:
                    flush_av(False)  # AV matmuls lag behind S for overlap
                drip_proj(2)
                pending_av.append(((r, j), d0, erow, vi))

            # drain remaining next-step projection matmuls, then its cast
            drip_proj(len(next_proj))
            if j > 0:
                proj_cast(j - 1, pproj_next)

        flush_av(True)


def _enable_ldw_opt():
    """Flip walrus's --enable-ldw-opt to true for our compile: consecutive
    matmuls reusing the same stationary operand then skip the reload."""
    import concourse.bass_utils as bu

    if getattr(bu, "_ldw_opt_patched", False):
        return
    orig = bu.run_command

    def run_command_ldw(cmd, *a, **kw):
        if isinstance(cmd, list):
            cmd = [
                "--enable-ldw-opt=true" if c == "--enable-ldw-opt=false" else c
                for c in cmd
            ]
        return orig(cmd, *a, **kw)

    bu.run_command = run_command_ldw
    bu._ldw_opt_patched = True


def _build_program():
    # NOTE: walrus rejects --enable-ldw-opt=true when the program contains
    # transpose-mode LDWEIGHTS (our pvt transposes), so this stays opt-in.
    if os.environ.get("BASS_LDW_OPT", "0") == "1":
        _enable_ldw_opt()
    nc = bacc.Bacc("TRN2", target_bir_lowering=False, debug=False, num_devices=B)
    xb_d = nc.dram_tensor("xb", [NJ, P, NE * CB], BF16, kind="ExternalInput").ap()
    wall_d = nc.dram_tensor("wall", [P, WALLW], BF16, kind="ExternalInput").ap()
    out_d = nc.dram_tensor("out", [P, 2 * CB], F32, kind="ExternalOutput").ap()
    with tile.TileContext(nc) as tc:
        _emit(tc, xb_d, wall_d, out_d)
    nc.compile()
    return nc


def _host_masks():
    """[128, 4*128] additive triangles: row r masks t < s within the
    diagonal 128-block (t-local f, partition p: keep f >= p)."""
    m = np.full((P, 4 * P), MASK_NEG, dtype=np.float32)
    p = np.arange(P)[:, None]
    f = np.arange(P)[None, :]
    for r in range(4):
        m[:, r * P : (r + 1) * P][f >= p] = 0.0
    return m


def _host_inputs(x, Wk, Wq, Wv):
    bf = ml_dtypes.bfloat16
    x = np.asarray(x, dtype=np.float32)
    # [B, E, T] -> block-major [B, NJ, P, NE*CB] so each block is one
    # contiguous DMA with 4KB/partition runs
    xT = np.transpose(x, (0, 2, 1)).reshape(B, NE, P, NJ, CB)
    xb = np.ascontiguousarray(xT.transpose(0, 3, 2, 1, 4)).reshape(
        B, NJ, P, NE * CB
    ).astype(bf)

    def chunks(w):  # [E, h] -> [NE, P, h]
        return np.asarray(w, np.float32).reshape(NE, P, -1)

    # wkv chunk e = [Wk_e | Wv_e] -> [P, NE*128]
    kv = np.concatenate([chunks(Wk), chunks(Wv)], axis=2)
    wkv = kv.transpose(1, 0, 2).reshape(P, NE * P)
    wqp = chunks(Wq).transpose(1, 0, 2).reshape(P, NE * H)
    wall = np.concatenate(
        [wkv, wqp, _host_masks(), np.eye(P, dtype=np.float32)], axis=1
    ).astype(bf)
    assert wall.shape == (P, WALLW)
    return [{"xb": xb[b], "wall": wall} for b in range(B)]


def _unpack_out(outT):
    """[128, 1024] out^T banks -> [T, H] natural layout."""
    o = np.empty((T, H), dtype=np.float32)
    for a in range(2):
        for h2 in range(2):
            jj = 2 * a + h2
            o[jj * CB : (jj + 1) * CB, :] = outT[
                H * h2 : H * (h2 + 1), a * CB : (a + 1) * CB
            ].T
    return o


def _ensure_axon_ntff_hook():
    """The agent image's antenv lacks axon_hooks; synthesize it so
    run_bass_kernel_spmd's trace path can find the NTFF profile hook."""
    import sys
    import types

    if "antenv.axon_hooks" in sys.modules:
        return
    try:
        import antenv

        mod = types.ModuleType("antenv.axon_hooks")
        mod._hook = None

        def set_axon_ntff_profile_hook(h):
            mod._hook = h

        def get_axon_ntff_profile_hook():
            return mod._hook

        mod.set_axon_ntff_profile_hook = set_axon_ntff_profile_hook
        mod.get_axon_ntff_profile_hook = get_axon_ntff_profile_hook
        sys.modules["antenv.axon_hooks"] = mod
        antenv.axon_hooks = mod

        from trn_agent_boot.trn_boot import _ntff_profile_via_ctypes

        hook = _ntff_profile_via_ctypes("/opt/axon/libaxon_pjrt.so")
        if hook is not None:
            mod._hook = hook
    except Exception as e:  # degrade to untraced run
        print(f"NTFF hook setup failed ({e}); tracing will be skipped")


def kernel(x, Wk, Wq, Wv, _trace=False, _trace_kwargs=None):
    if _trace:
        _ensure_axon_ntff_hook()
    in_maps = _host_inputs(x, Wk, Wq, Wv)
    nc = _build_program()
    res = bass_utils.run_bass_kernel_spmd(
        nc, in_maps, list(range(B)), trace=_trace, **(_trace_kwargs or {})
    )
    out = np.stack(
        [_unpack_out(res.results[b]["out"]) for b in range(B)], axis=0
    )
    if _trace:
        kernel.last_results = res
    return out.astype(np.float32)
